# revision 22
# baseline (speedup 1.0000x reference)
"""Causal single-head attention (B=8, T=2048, C=1024, H=64) on 8 trn2 NeuronCores.

Strategy (data-parallel over batch, one batch element per core):
  host: feed xT = x[b].T (so C is the partition/contraction dim on chip),
        wqk = [Wq/sqrt(C) | Wk] fused projection weight, wv = [Wv | Wv] (dup for
        column-tiled V projection).
  core, per q-block of 512 tokens:
    passA: one fused matmul chain per C-chunk -> psum rows 0:64 = qT block,
           rows 64:128 = kT block.  Copied (4x) into row-duplicated SBUF tiles
           qT2/kT2 so QK stationaries can alternate PE row-halves.
    passB: column-tiled pair (tile_position (0,0)/(0,64)) -> vT block; PE
           transpose -> v[s, 64]; ones column appended (softmax denominator
           accumulates through the PV matmul's 65th output row).
    QK:   scores^T[s,q] per 128-wide s-chunk; stationary kT chunk alternates
          row-half 0/1 with chunk parity so each LDWEIGHTS overlaps the
          previous chunk's MATMUL (no row-group conflict).
    exp:  ACT reads two adjacent psum banks [128,1024] at once -> pT (fp32r).
          No max subtraction: |scores| < ~1 so exp is exact-safe and equals
          softmax numerator exactly.
    mask: zero cols < d, triangular 128x128 multiply on the diagonal square.
    PV:   out_aug^T[65, q] += v_aug^T-stationary @ pT-moving.
  host: out = (out_aug[:64] / out_aug[64]).T, stack cores.

All matmuls in float32r: full PE rate at moving-N>=256, ~2e-4 rel err.
"""

import numpy as np

import concourse.bass as bass
import concourse.mybir as mybir
import concourse.tile as tile
from concourse import bacc
from concourse.bass_utils import run_bass_kernel_spmd

B, T, C, H = 8, 2048, 1024, 64
TB = 512                 # q-block width
NBLK = T // TB           # 4 q-blocks
NC = C // 128            # 8 contraction chunks
NSC = T // 128           # 16 s-chunks
HA = H + 1               # v augmented with ones column
F32 = mybir.dt.float32
FR = mybir.dt.float32r

_compiled = {}


def build_nc():
    nc = bacc.Bacc("TRN2", target_bir_lowering=False, debug=False, num_devices=8)

    xT_d = nc.dram_tensor("xT", [C, T], FR, kind="ExternalInput").ap()
    wqk_d = nc.dram_tensor("wqk", [C, 128], FR, kind="ExternalInput").ap()
    wv_d = nc.dram_tensor("wv", [C, 128], FR, kind="ExternalInput").ap()
    # col 0:128 = causal upper-triangle mask, col 128 = ones
    tri_d = nc.dram_tensor("tri", [128, 129], FR, kind="ExternalInput").ap()
    outT_d = nc.dram_tensor("outT", [HA, T], F32, kind="ExternalOutput").ap()

    xT_r = xT_d.rearrange("(co ci) t -> ci co t", ci=128)
    wqk_r = wqk_d.rearrange("(co ci) m -> ci co m", ci=128)
    wv_r = wv_d.rearrange("(co ci) m -> ci co m", ci=128)

    with tile.TileContext(nc) as tc:
        with (
            tc.tile_pool(name="const", bufs=1) as cpool,
            tc.tile_pool(name="persist", bufs=1) as ppool,
            tc.tile_pool(name="xin", bufs=18) as xpool,
            tc.tile_pool(name="ptile", bufs=3) as pt_pool,
            tc.tile_pool(name="vtmp", bufs=2) as vtmp_pool,
            tc.tile_pool(name="outsb", bufs=2) as out_pool,
            tc.tile_pool(name="psA", bufs=2, space="PSUM") as psA_pool,
            tc.tile_pool(name="psB", bufs=1, space="PSUM") as psB_pool,
            tc.tile_pool(name="psQK", bufs=2, space="PSUM") as psQK_pool,
            tc.tile_pool(name="psO", bufs=1, space="PSUM") as psO_pool,
        ):
            # constants / weights on the scalar HWDGE ring so they don't queue
            # behind the x stream on the sync ring
            wqk_s = cpool.tile([128, NC, 128], FR)
            wv_s = cpool.tile([128, NC, 128], FR)
            tri_full = cpool.tile([128, 129], FR)
            tri_s = tri_full[:, 0:128]
            ones_s = tri_full[:, 128:129]
            ident = cpool.tile([128, 64], F32)
            # chunk 0 first so the very first matmul's weights land ASAP
            nc.scalar.dma_start(wqk_s[:, 0:1, :], wqk_r[:, 0:1, :])
            nc.scalar.dma_start(wqk_s[:, 1:NC, :], wqk_r[:, 1:NC, :])
            nc.scalar.dma_start(wv_s[:], wv_r[:])
            nc.scalar.dma_start(tri_full[:], tri_d[:])

            # PE warm-up: dummy self-contained matmuls with no DMA deps keep
            # the PE busy through the initial DMA wait so the HAM clock gate
            # reaches K=8/8 before real work arrives.
            warm_w = cpool.tile([128, 128], F32)
            warm_x = cpool.tile([128, 512], F32)
            nc.vector.memset(warm_w[:], 0.0)
            nc.vector.memset(warm_x[:], 0.0)

            def warm_mm(n):
                ps_warm = psQK_pool.tile([128, 512], F32, tag="psQK")
                nc.tensor.matmul(ps_warm[:, 0:n], warm_w[:], warm_x[:, 0:n],
                                 start=True, stop=True)

            # bridge the initial DMA wait (fp32 = 2 passes, long dense ops)
            for w in range(4):
                warm_mm(512)
            for h2 in range(2):
                sl = ident[h2 * 64 : (h2 + 1) * 64, :]
                nc.gpsimd.memset(sl, 0.0)
                nc.gpsimd.affine_select(
                    out=sl,
                    in_=sl,
                    compare_op=mybir.AluOpType.not_equal,
                    fill=1.0,
                    base=0,
                    pattern=[[-1, 64]],
                    channel_multiplier=1,
                )

            # row-duplicated q/k (rows 0:64 == rows 64:128) for row-half
            # alternating QK stationaries/movings
            qT2_s = ppool.tile([128, T], FR)
            kT2_s = ppool.tile([128, T], FR)
            v_s = ppool.tile([128, NSC * HA], FR)

            for i in range(NBLK):
                q0 = i * TB
                # ---- x DMA for this block (sync ring) ----
                x_c = []
                for c in range(NC):
                    xc = xpool.tile([128, TB], FR)
                    if i == 0 and c == 0:
                        # split so the first 128 cols land (and the first
                        # matmul can start) as early as possible
                        nc.sync.dma_start(xc[:, 0:128], xT_r[:, c, q0 : q0 + 128])
                        nc.sync.dma_start(
                            xc[:, 128:TB], xT_r[:, c, q0 + 128 : q0 + TB]
                        )
                    else:
                        nc.sync.dma_start(xc[:], xT_r[:, c, q0 : q0 + TB])
                    x_c.append(xc)

                # ---- passA+passB interleaved per C-chunk (q/k and v
                # projections both consume x_c right as it lands; the pair of
                # matmuls keeps the PE dense inside the DMA-paced window) ----
                psA = psA_pool.tile([128, TB], F32)
                psB = psB_pool.tile([64, TB], F32)
                for c in range(NC):
                    if i == 0 and c == 0:
                        # start=True clears the whole bank; the second piece
                        # (start=False) overwrites its untouched columns
                        nc.tensor.matmul(
                            psA[:, 0:128], wqk_s[:, 0, :], x_c[0][:, 0:128],
                            start=True, stop=False,
                        )
                        nc.tensor.matmul(
                            psA[:, 128:TB], wqk_s[:, 0, :], x_c[0][:, 128:TB],
                            start=False, stop=False,
                        )
                    else:
                        nc.tensor.matmul(
                            psA[:], wqk_s[:, c, :], x_c[c][:],
                            start=(c == 0 and i != 0), stop=(c == NC - 1),
                        )
                    nc.tensor.matmul(
                        psB[:], wv_s[:, c, 0:64], x_c[c][:],
                        start=(c == 0), stop=(c == NC - 1),
                    )
                for h2 in range(2):
                    r = slice(h2 * 64, h2 * 64 + 64)
                    nc.vector.tensor_copy(qT2_s[r, q0 : q0 + TB], psA[0:64, :])
                    nc.vector.tensor_copy(kT2_s[r, q0 : q0 + TB], psA[64:128, :])

                vT_tmp = vtmp_pool.tile([64, TB], F32)
                nc.vector.tensor_copy(vT_tmp[:], psB[:])
                for j4 in range(TB // 128):
                    sj = (TB // 128) * i + j4
                    ps_vt = psQK_pool.tile([128, 64], F32, tag="psQK")
                    nc.tensor.transpose(
                        ps_vt[:],
                        vT_tmp[:, j4 * 128 : (j4 + 1) * 128],
                        ident[0:64, :],
                    )
                    nc.vector.tensor_copy(v_s[:, sj * HA : sj * HA + H], ps_vt[:])
                    nc.vector.tensor_copy(
                        v_s[:, sj * HA + H : sj * HA + HA], ones_s[:]
                    )

                # ---- attention for this q-block ----
                nsc_i = (TB // 128) * (i + 1)  # s-chunks 0..nsc_i-1 (causal)
                psO = psO_pool.tile([HA, TB], F32)
                for g in range(nsc_i // 2):  # pairs of s-chunks share one exp
                    # lo = first causally-valid q column for each chunk;
                    # QK/exp/PV all restrict to [lo, TB) (cols < lo contribute 0)
                    js = [2 * g, 2 * g + 1]
                    ds = [j * 128 - q0 for j in js]
                    los = [max(dd, 0) for dd in ds]
                    psQK = psQK_pool.tile([128, 1024], F32, tag="psQK")
                    for h2 in range(2):
                        j, lo = js[h2], los[h2]
                        r = slice(h2 * 64, h2 * 64 + 64)  # alternate row halves
                        nc.tensor.matmul(
                            psQK[:, h2 * 512 + lo : (h2 + 1) * 512],
                            kT2_s[r, j * 128 : (j + 1) * 128],
                            qT2_s[r, q0 + lo : q0 + TB],
                            start=True, stop=True,
                        )
                    pT = pt_pool.tile([128, 1024], FR)
                    if los[0] == 0 and los[1] == 0:
                        nc.scalar.activation(
                            pT[:], psQK[:], mybir.ActivationFunctionType.Exp
                        )
                    else:
                        for h2 in range(2):
                            lo = los[h2]
                            sl = slice(h2 * 512 + lo, (h2 + 1) * 512)
                            nc.scalar.activation(
                                pT[:, sl], psQK[:, sl],
                                mybir.ActivationFunctionType.Exp,
                            )
                    for h2 in range(2):
                        j, d, lo = js[h2], ds[h2], los[h2]
                        pj = pT[:, h2 * 512 : (h2 + 1) * 512]
                        if d >= 0:  # diagonal chunk: triangular causal mask
                            nc.vector.tensor_mul(
                                pj[:, d : d + 128], pj[:, d : d + 128], tri_s[:]
                            )
                        nc.tensor.matmul(
                            psO[:, lo:TB],
                            v_s[:, j * HA : (j + 1) * HA],
                            pj[:, lo:TB],
                            start=(j == 0), stop=(j == nsc_i - 1),
                        )
                out_sb = out_pool.tile([HA, TB], F32)
                if i < NBLK - 1:
                    # ACT copy (DVE stays free); SWDGE DMA so the sync HWDGE
                    # ring isn't head-of-line-blocked for the x stream
                    nc.scalar.copy(out_sb[:], psO[:])
                    nc.gpsimd.dma_start(outT_d[:, q0 : q0 + TB], out_sb[:])
                else:
                    # tail: two halves so the first DMA overlaps the second
                    # copy; sync ring is idle by now
                    for h2 in range(2):
                        sl = slice(h2 * (TB // 2), (h2 + 1) * (TB // 2))
                        nc.vector.tensor_copy(out_sb[:, sl], psO[:, sl])
                        nc.sync.dma_start(
                            outT_d[:, q0 + h2 * (TB // 2) : q0 + (h2 + 1) * (TB // 2)],
                            out_sb[:, sl],
                        )

    nc.compile()
    return nc


def _get_nc():
    if "nc" not in _compiled:
        _compiled["nc"] = build_nc()
    return _compiled["nc"]


def make_in_maps(x, Wk, Wq, Wv):
    x = np.asarray(x, dtype=np.float32)
    Wk = np.asarray(Wk, dtype=np.float32)
    Wq = np.asarray(Wq, dtype=np.float32)
    Wv = np.asarray(Wv, dtype=np.float32)
    scale = np.float32(1.0 / np.sqrt(np.float32(C)))
    wqk = np.concatenate([Wq * scale, Wk], axis=1)  # [C, 128]
    wvd = np.concatenate([Wv, Wv], axis=1)          # [C, 128] duplicated
    tri = np.ones((128, 129), dtype=np.float32)
    tri[:, 0:128] = np.triu(np.ones((128, 128), dtype=np.float32))
    in_maps = []
    for b in range(B):
        in_maps.append(
            {
                "xT": np.ascontiguousarray(x[b].T),
                "wqk": wqk,
                "wv": wvd,
                "tri": tri,
            }
        )
    return in_maps


def postprocess(results):
    outs = []
    for b in range(B):
        outT = results[b]["outT"]  # [65, T]
        out = (outT[:H] / outT[H : H + 1]).T  # [T, H]
        outs.append(out)
    return np.stack(outs).astype(np.float32)


def run(x, Wk, Wq, Wv, trace=False, **kw):
    nc = _get_nc()
    in_maps = make_in_maps(x, Wk, Wq, Wv)
    res = run_bass_kernel_spmd(
        nc, in_maps, core_ids=list(range(B)), trace=trace, **kw
    )
    return postprocess(res.results), res


def kernel(x, Wk, Wq, Wv):
    out, _ = run(x, Wk, Wq, Wv, trace=False)
    return out


# revision 26
# speedup vs baseline: 1.1387x; 1.1387x over previous
"""Causal single-head attention (B=8, T=2048, C=1024, H=64) on 8 trn2 NeuronCores.

Strategy (data-parallel over batch, one batch element per core):
  host: feed xT = x[b].T (so C is the partition/contraction dim on chip),
        wqk = [Wq/sqrt(C) | Wk] fused projection weight, wv = [Wv | Wv] (dup for
        column-tiled V projection).
  core, per q-block of 512 tokens:
    passA: one fused matmul chain per C-chunk -> psum rows 0:64 = qT block,
           rows 64:128 = kT block.  Copied (4x) into row-duplicated SBUF tiles
           qT2/kT2 so QK stationaries can alternate PE row-halves.
    passB: column-tiled pair (tile_position (0,0)/(0,64)) -> vT block; PE
           transpose -> v[s, 64]; ones column appended (softmax denominator
           accumulates through the PV matmul's 65th output row).
    QK:   scores^T[s,q] per 128-wide s-chunk; stationary kT chunk alternates
          row-half 0/1 with chunk parity so each LDWEIGHTS overlaps the
          previous chunk's MATMUL (no row-group conflict).
    exp:  ACT reads two adjacent psum banks [128,1024] at once -> pT (fp32r).
          No max subtraction: |scores| < ~1 so exp is exact-safe and equals
          softmax numerator exactly.
    mask: zero cols < d, triangular 128x128 multiply on the diagonal square.
    PV:   out_aug^T[65, q] += v_aug^T-stationary @ pT-moving.
  host: out = (out_aug[:64] / out_aug[64]).T, stack cores.

All matmuls in float32r: full PE rate at moving-N>=256, ~2e-4 rel err.
"""

import numpy as np

import concourse.bass as bass
import concourse.mybir as mybir
import concourse.tile as tile
from concourse import bacc
from concourse.bass_utils import run_bass_kernel_spmd

B, T, C, H = 8, 2048, 1024, 64
TB = 512                 # q-block width
NBLK = T // TB           # 4 q-blocks
NC = C // 128            # 8 contraction chunks
NSC = T // 128           # 16 s-chunks
HA = H + 1               # v augmented with ones column
F32 = mybir.dt.float32
F16 = mybir.dt.float16

_compiled = {}


def build_nc():
    nc = bacc.Bacc("TRN2", target_bir_lowering=False, debug=False, num_devices=8)

    xT_d = nc.dram_tensor("xT", [C, T], F16, kind="ExternalInput").ap()
    wqk_d = nc.dram_tensor("wqk", [C, 128], F16, kind="ExternalInput").ap()
    wv_d = nc.dram_tensor("wv", [C, 128], F16, kind="ExternalInput").ap()
    # col 0:128 = causal upper-triangle mask, col 128 = ones
    tri_d = nc.dram_tensor("tri", [128, 129], F16, kind="ExternalInput").ap()
    outT_d = nc.dram_tensor("outT", [HA, T], F32, kind="ExternalOutput").ap()

    xT_r = xT_d.rearrange("(co ci) t -> ci co t", ci=128)
    wqk_r = wqk_d.rearrange("(co ci) m -> ci co m", ci=128)
    wv_r = wv_d.rearrange("(co ci) m -> ci co m", ci=128)

    with tile.TileContext(nc) as tc:
        with (
            tc.tile_pool(name="const", bufs=1) as cpool,
            tc.tile_pool(name="persist", bufs=1) as ppool,
            tc.tile_pool(name="xin", bufs=18) as xpool,
            tc.tile_pool(name="ptile", bufs=3) as pt_pool,
            tc.tile_pool(name="vtmp", bufs=2) as vtmp_pool,
            tc.tile_pool(name="outsb", bufs=2) as out_pool,
            tc.tile_pool(name="psA", bufs=2, space="PSUM") as psA_pool,
            tc.tile_pool(name="psB", bufs=1, space="PSUM") as psB_pool,
            tc.tile_pool(name="psQK", bufs=2, space="PSUM") as psQK_pool,
            tc.tile_pool(name="psO", bufs=1, space="PSUM") as psO_pool,
        ):
            # constants / weights on the scalar HWDGE ring so they don't queue
            # behind the x stream on the sync ring
            wqk_s = cpool.tile([128, NC, 128], F16)
            wv_s = cpool.tile([128, NC, 128], F16)
            tri_full = cpool.tile([128, 129], F16)
            tri_s = tri_full[:, 0:128]
            ones_s = tri_full[:, 128:129]
            ident = cpool.tile([128, 64], F32)
            # chunk 0 first so the very first matmul's weights land ASAP
            nc.scalar.dma_start(wqk_s[:, 0:1, :], wqk_r[:, 0:1, :])
            nc.scalar.dma_start(wqk_s[:, 1:NC, :], wqk_r[:, 1:NC, :])
            nc.scalar.dma_start(wv_s[:], wv_r[:])
            nc.scalar.dma_start(tri_full[:], tri_d[:])

            # PE warm-up: dummy self-contained matmuls with no DMA deps keep
            # the PE busy through the initial DMA wait so the HAM clock gate
            # reaches K=8/8 before real work arrives.
            warm_w = cpool.tile([128, 128], F32)
            warm_x = cpool.tile([128, 512], F32)
            nc.vector.memset(warm_w[:], 0.0)
            nc.vector.memset(warm_x[:], 0.0)

            def warm_mm(n):
                ps_warm = psQK_pool.tile([128, 512], F32, tag="psQK")
                nc.tensor.matmul(ps_warm[:, 0:n], warm_w[:], warm_x[:, 0:n],
                                 start=True, stop=True)

            # bridge the initial DMA wait (fp32 = 2 passes, long dense ops)
            for w in range(4):
                warm_mm(512)
            for h2 in range(2):
                sl = ident[h2 * 64 : (h2 + 1) * 64, :]
                nc.gpsimd.memset(sl, 0.0)
                nc.gpsimd.affine_select(
                    out=sl,
                    in_=sl,
                    compare_op=mybir.AluOpType.not_equal,
                    fill=1.0,
                    base=0,
                    pattern=[[-1, 64]],
                    channel_multiplier=1,
                )

            # row-duplicated q/k (rows 0:64 == rows 64:128) for row-half
            # alternating QK stationaries/movings
            qT2_s = ppool.tile([128, T], F16)
            kT2_s = ppool.tile([128, T], F16)
            v_s = ppool.tile([128, NSC * HA], F16)

            for i in range(NBLK):
                q0 = i * TB
                # ---- x DMA for this block (sync ring) ----
                x_c = []
                for c in range(NC):
                    xc = xpool.tile([128, TB], F16)
                    if i == 0 and c == 0:
                        # split so the first 128 cols land (and the first
                        # matmul can start) as early as possible
                        nc.sync.dma_start(xc[:, 0:128], xT_r[:, c, q0 : q0 + 128])
                        nc.sync.dma_start(
                            xc[:, 128:TB], xT_r[:, c, q0 + 128 : q0 + TB]
                        )
                    else:
                        nc.sync.dma_start(xc[:], xT_r[:, c, q0 : q0 + TB])
                    x_c.append(xc)

                # ---- passA+passB interleaved per C-chunk (q/k and v
                # projections both consume x_c right as it lands; the pair of
                # matmuls keeps the PE dense inside the DMA-paced window) ----
                psA = psA_pool.tile([128, TB], F32)
                psB = psB_pool.tile([64, TB], F32)
                for c in range(NC):
                    if i == 0 and c == 0:
                        # start=True clears the whole bank; the second piece
                        # (start=False) overwrites its untouched columns
                        nc.tensor.matmul(
                            psA[:, 0:128], wqk_s[:, 0, :], x_c[0][:, 0:128],
                            start=True, stop=False,
                        )
                        nc.tensor.matmul(
                            psA[:, 128:TB], wqk_s[:, 0, :], x_c[0][:, 128:TB],
                            start=False, stop=False,
                        )
                    else:
                        nc.tensor.matmul(
                            psA[:], wqk_s[:, c, :], x_c[c][:],
                            start=(c == 0 and i != 0), stop=(c == NC - 1),
                        )
                    nc.tensor.matmul(
                        psB[:], wv_s[:, c, 0:64], x_c[c][:],
                        start=(c == 0), stop=(c == NC - 1),
                    )
                for h2 in range(2):
                    r = slice(h2 * 64, h2 * 64 + 64)
                    nc.vector.tensor_copy(qT2_s[r, q0 : q0 + TB], psA[0:64, :])
                    nc.vector.tensor_copy(kT2_s[r, q0 : q0 + TB], psA[64:128, :])

                vT_tmp = vtmp_pool.tile([64, TB], F32)
                nc.vector.tensor_copy(vT_tmp[:], psB[:])
                for j4 in range(TB // 128):
                    sj = (TB // 128) * i + j4
                    ps_vt = psQK_pool.tile([128, 64], F32, tag="psQK")
                    nc.tensor.transpose(
                        ps_vt[:],
                        vT_tmp[:, j4 * 128 : (j4 + 1) * 128],
                        ident[0:64, :],
                    )
                    nc.vector.tensor_copy(v_s[:, sj * HA : sj * HA + H], ps_vt[:])
                    nc.vector.tensor_copy(
                        v_s[:, sj * HA + H : sj * HA + HA], ones_s[:]
                    )

                # ---- attention for this q-block ----
                nsc_i = (TB // 128) * (i + 1)  # s-chunks 0..nsc_i-1 (causal)
                psO = psO_pool.tile([HA, TB], F32)
                for g in range(nsc_i // 2):  # pairs of s-chunks share one exp
                    # lo = first causally-valid q column for each chunk;
                    # QK/exp/PV all restrict to [lo, TB) (cols < lo contribute 0)
                    js = [2 * g, 2 * g + 1]
                    ds = [j * 128 - q0 for j in js]
                    los = [max(dd, 0) for dd in ds]
                    psQK = psQK_pool.tile([128, 1024], F32, tag="psQK")
                    for h2 in range(2):
                        j, lo = js[h2], los[h2]
                        r = slice(h2 * 64, h2 * 64 + 64)  # alternate row halves
                        nc.tensor.matmul(
                            psQK[:, h2 * 512 + lo : (h2 + 1) * 512],
                            kT2_s[r, j * 128 : (j + 1) * 128],
                            qT2_s[r, q0 + lo : q0 + TB],
                            start=True, stop=True,
                        )
                    # scores are raw q.k here; the 1/sqrt(C) softmax scale is
                    # applied through ACT's free affine pre-scale (keeping it
                    # out of Wq avoids fp16-subnormal weights)
                    pT = pt_pool.tile([128, 1024], F16)
                    if los[0] == 0 and los[1] == 0:
                        nc.scalar.activation(
                            pT[:], psQK[:], mybir.ActivationFunctionType.Exp,
                            scale=float(1.0 / np.sqrt(C)),
                        )
                    else:
                        for h2 in range(2):
                            lo = los[h2]
                            sl = slice(h2 * 512 + lo, (h2 + 1) * 512)
                            nc.scalar.activation(
                                pT[:, sl], psQK[:, sl],
                                mybir.ActivationFunctionType.Exp,
                                scale=float(1.0 / np.sqrt(C)),
                            )
                    for h2 in range(2):
                        j, d, lo = js[h2], ds[h2], los[h2]
                        pj = pT[:, h2 * 512 : (h2 + 1) * 512]
                        if d >= 0:  # diagonal chunk: triangular causal mask
                            nc.vector.tensor_mul(
                                pj[:, d : d + 128], pj[:, d : d + 128], tri_s[:]
                            )
                        nc.tensor.matmul(
                            psO[:, lo:TB],
                            v_s[:, j * HA : (j + 1) * HA],
                            pj[:, lo:TB],
                            start=(j == 0), stop=(j == nsc_i - 1),
                        )
                out_sb = out_pool.tile([HA, TB], F32)
                if i < NBLK - 1:
                    # ACT copy (DVE stays free); SWDGE DMA so the sync HWDGE
                    # ring isn't head-of-line-blocked for the x stream
                    nc.scalar.copy(out_sb[:], psO[:])
                    nc.gpsimd.dma_start(outT_d[:, q0 : q0 + TB], out_sb[:])
                else:
                    # tail: two halves so the first DMA overlaps the second
                    # copy; sync ring is idle by now
                    for h2 in range(2):
                        sl = slice(h2 * (TB // 2), (h2 + 1) * (TB // 2))
                        nc.vector.tensor_copy(out_sb[:, sl], psO[:, sl])
                        nc.sync.dma_start(
                            outT_d[:, q0 + h2 * (TB // 2) : q0 + (h2 + 1) * (TB // 2)],
                            out_sb[:, sl],
                        )

    nc.compile()
    return nc


def _get_nc():
    if "nc" not in _compiled:
        _compiled["nc"] = build_nc()
    return _compiled["nc"]


def make_in_maps(x, Wk, Wq, Wv):
    x = np.asarray(x, dtype=np.float32)
    Wk = np.asarray(Wk, dtype=np.float32)
    Wq = np.asarray(Wq, dtype=np.float32)
    Wv = np.asarray(Wv, dtype=np.float32)
    # raw Wq (no 1/sqrt(C) here — that scale rides the exp's affine pre-scale)
    wqk = np.concatenate([Wq, Wk], axis=1).astype(np.float16)  # [C, 128]
    wvd = np.concatenate([Wv, Wv], axis=1).astype(np.float16)  # [C, 128] dup
    tri = np.ones((128, 129), dtype=np.float16)
    tri[:, 0:128] = np.triu(np.ones((128, 128), dtype=np.float16))
    in_maps = []
    for b in range(B):
        in_maps.append(
            {
                "xT": np.ascontiguousarray(x[b].T.astype(np.float16)),
                "wqk": wqk,
                "wv": wvd,
                "tri": tri,
            }
        )
    return in_maps


def postprocess(results):
    outs = []
    for b in range(B):
        outT = results[b]["outT"]  # [65, T]
        out = (outT[:H] / outT[H : H + 1]).T  # [T, H]
        outs.append(out)
    return np.stack(outs).astype(np.float32)


def run(x, Wk, Wq, Wv, trace=False, **kw):
    nc = _get_nc()
    in_maps = make_in_maps(x, Wk, Wq, Wv)
    res = run_bass_kernel_spmd(
        nc, in_maps, core_ids=list(range(B)), trace=trace, **kw
    )
    return postprocess(res.results), res


def kernel(x, Wk, Wq, Wv):
    out, _ = run(x, Wk, Wq, Wv, trace=False)
    return out


# revision 31
# speedup vs baseline: 1.1578x; 1.0168x over previous
"""Causal single-head attention (B=8, T=2048, C=1024, H=64) on 8 trn2 NeuronCores.

Strategy (data-parallel over batch, one batch element per core):
  host: feed xT = x[b].T (so C is the partition/contraction dim on chip),
        wqk = [Wq/sqrt(C) | Wk] fused projection weight, wv = [Wv | Wv] (dup for
        column-tiled V projection).
  core, per q-block of 512 tokens:
    passA: one fused matmul chain per C-chunk -> psum rows 0:64 = qT block,
           rows 64:128 = kT block.  Copied (4x) into row-duplicated SBUF tiles
           qT2/kT2 so QK stationaries can alternate PE row-halves.
    passB: column-tiled pair (tile_position (0,0)/(0,64)) -> vT block; PE
           transpose -> v[s, 64]; ones column appended (softmax denominator
           accumulates through the PV matmul's 65th output row).
    QK:   scores^T[s,q] per 128-wide s-chunk; stationary kT chunk alternates
          row-half 0/1 with chunk parity so each LDWEIGHTS overlaps the
          previous chunk's MATMUL (no row-group conflict).
    exp:  ACT reads two adjacent psum banks [128,1024] at once -> pT (fp32r).
          No max subtraction: |scores| < ~1 so exp is exact-safe and equals
          softmax numerator exactly.
    mask: zero cols < d, triangular 128x128 multiply on the diagonal square.
    PV:   out_aug^T[65, q] += v_aug^T-stationary @ pT-moving.
  host: out = (out_aug[:64] / out_aug[64]).T, stack cores.

All matmuls in float32r: full PE rate at moving-N>=256, ~2e-4 rel err.
"""

import numpy as np

import concourse.bass as bass
import concourse.mybir as mybir
import concourse.tile as tile
from concourse import bacc
from concourse.bass_utils import run_bass_kernel_spmd

B, T, C, H = 8, 2048, 1024, 64
TB = 512                 # q-block width
NBLK = T // TB           # 4 q-blocks
NC = C // 128            # 8 contraction chunks
NSC = T // 128           # 16 s-chunks
HA = H + 1               # v augmented with ones column
F32 = mybir.dt.float32
F16 = mybir.dt.float16

_compiled = {}


def build_nc():
    nc = bacc.Bacc("TRN2", target_bir_lowering=False, debug=False, num_devices=8)

    xT_d = nc.dram_tensor("xT", [C, T], F16, kind="ExternalInput").ap()
    wqk_d = nc.dram_tensor("wqk", [C, 128], F16, kind="ExternalInput").ap()
    wv_d = nc.dram_tensor("wv", [C, 128], F16, kind="ExternalInput").ap()
    # col 0:128 = causal upper-triangle mask, col 128 = ones
    tri_d = nc.dram_tensor("tri", [128, 129], F16, kind="ExternalInput").ap()
    outT_d = nc.dram_tensor("outT", [HA, T], F32, kind="ExternalOutput").ap()

    xT_r = xT_d.rearrange("(co ci) t -> ci co t", ci=128)
    wqk_r = wqk_d.rearrange("(co ci) m -> ci co m", ci=128)
    wv_r = wv_d.rearrange("(co ci) m -> ci co m", ci=128)

    with tile.TileContext(nc) as tc:
        with (
            tc.tile_pool(name="const", bufs=1) as cpool,
            tc.tile_pool(name="persist", bufs=1) as ppool,
            tc.tile_pool(name="xin", bufs=24) as xpool,
            tc.tile_pool(name="ptile", bufs=4) as pt_pool,
            tc.tile_pool(name="vtmp", bufs=2) as vtmp_pool,
            tc.tile_pool(name="outsb", bufs=2) as out_pool,
            tc.tile_pool(name="psA", bufs=2, space="PSUM") as psA_pool,
            tc.tile_pool(name="psB", bufs=1, space="PSUM") as psB_pool,
            tc.tile_pool(name="psQK", bufs=2, space="PSUM") as psQK_pool,
            tc.tile_pool(name="psO", bufs=1, space="PSUM") as psO_pool,
        ):
            # constants / weights on the scalar HWDGE ring so they don't queue
            # behind the x stream on the sync ring
            wqk_s = cpool.tile([128, NC, 128], F16)
            wv_s = cpool.tile([128, NC, 128], F16)
            tri_full = cpool.tile([128, 129], F16)
            tri_s = tri_full[:, 0:128]
            ones_s = tri_full[:, 128:129]
            ident = cpool.tile([128, 64], F16)
            # chunk 0 first so the very first matmul's weights land ASAP
            nc.scalar.dma_start(wqk_s[:, 0:1, :], wqk_r[:, 0:1, :])
            nc.scalar.dma_start(wqk_s[:, 1:NC, :], wqk_r[:, 1:NC, :])
            nc.scalar.dma_start(wv_s[:], wv_r[:])
            nc.scalar.dma_start(tri_full[:], tri_d[:])

            # PE warm-up: dummy self-contained matmuls with no DMA deps keep
            # the PE busy through the initial DMA wait so the HAM clock gate
            # reaches K=8/8 before real work arrives.
            warm_w = cpool.tile([128, 128], F32)
            warm_x = cpool.tile([128, 512], F32)
            nc.vector.memset(warm_w[:], 0.0)
            nc.vector.memset(warm_x[:], 0.0)

            def warm_mm(n):
                ps_warm = psQK_pool.tile([128, 512], F32, tag="psQK")
                nc.tensor.matmul(ps_warm[:, 0:n], warm_w[:], warm_x[:, 0:n],
                                 start=True, stop=True)

            # bridge the initial DMA wait (fp32 = 2 passes, long dense ops)
            for w in range(4):
                warm_mm(512)
            for h2 in range(2):
                sl = ident[h2 * 64 : (h2 + 1) * 64, :]
                nc.gpsimd.memset(sl, 0.0)
                nc.gpsimd.affine_select(
                    out=sl,
                    in_=sl,
                    compare_op=mybir.AluOpType.not_equal,
                    fill=1.0,
                    base=0,
                    pattern=[[-1, 64]],
                    channel_multiplier=1,
                )

            # row-duplicated q/k (rows 0:64 == rows 64:128) for row-half
            # alternating QK stationaries/movings
            qT2_s = ppool.tile([128, T], F16)
            kT2_s = ppool.tile([128, T], F16)
            v_s = ppool.tile([128, NSC * HA], F16)

            for i in range(NBLK):
                q0 = i * TB
                # ---- x DMA for this block (sync ring) ----
                x_c = []
                for c in range(NC):
                    xc = xpool.tile([128, TB], F16)
                    if i == 0 and c == 0:
                        # split so the first 128 cols land (and the first
                        # matmul can start) as early as possible
                        nc.sync.dma_start(xc[:, 0:128], xT_r[:, c, q0 : q0 + 128])
                        nc.sync.dma_start(
                            xc[:, 128:TB], xT_r[:, c, q0 + 128 : q0 + TB]
                        )
                    else:
                        nc.sync.dma_start(xc[:], xT_r[:, c, q0 : q0 + TB])
                    x_c.append(xc)

                # ---- passA+passB interleaved per C-chunk (q/k and v
                # projections both consume x_c right as it lands; the pair of
                # matmuls keeps the PE dense inside the DMA-paced window) ----
                psA = psA_pool.tile([128, TB], F32)
                psB = psB_pool.tile([64, TB], F32)
                for c in range(NC):
                    if i == 0 and c == 0:
                        # start=True clears the whole bank; the second piece
                        # (start=False) overwrites its untouched columns
                        nc.tensor.matmul(
                            psA[:, 0:128], wqk_s[:, 0, :], x_c[0][:, 0:128],
                            start=True, stop=False,
                        )
                        nc.tensor.matmul(
                            psA[:, 128:TB], wqk_s[:, 0, :], x_c[0][:, 128:TB],
                            start=False, stop=False,
                        )
                    else:
                        nc.tensor.matmul(
                            psA[:], wqk_s[:, c, :], x_c[c][:],
                            start=(c == 0 and i != 0), stop=(c == NC - 1),
                        )
                    nc.tensor.matmul(
                        psB[:], wv_s[:, c, 0:64], x_c[c][:],
                        start=(c == 0), stop=(c == NC - 1),
                    )
                for h2 in range(2):
                    r = slice(h2 * 64, h2 * 64 + 64)
                    nc.vector.tensor_copy(qT2_s[r, q0 : q0 + TB], psA[0:64, :])
                    nc.vector.tensor_copy(kT2_s[r, q0 : q0 + TB], psA[64:128, :])

                vT_tmp = vtmp_pool.tile([64, TB], F16)
                nc.vector.tensor_copy(vT_tmp[:], psB[:])
                for j4 in range(TB // 128):
                    sj = (TB // 128) * i + j4
                    ps_vt = psQK_pool.tile([128, 64], F16, tag="psQK")
                    nc.tensor.transpose(
                        ps_vt[:],
                        vT_tmp[:, j4 * 128 : (j4 + 1) * 128],
                        ident[0:64, :],
                    )
                    nc.vector.tensor_copy(v_s[:, sj * HA : sj * HA + H], ps_vt[:])
                    nc.vector.tensor_copy(
                        v_s[:, sj * HA + H : sj * HA + HA], ones_s[:]
                    )

                # ---- attention for this q-block ----
                nsc_i = (TB // 128) * (i + 1)  # s-chunks 0..nsc_i-1 (causal)
                psO = psO_pool.tile([HA, TB], F32)
                if i == NBLK - 1:
                    out_sb_last = out_pool.tile([HA, TB], F32)
                for g in range(nsc_i // 2):  # pairs of s-chunks share one exp
                    # lo = first causally-valid q column for each chunk;
                    # QK/exp/PV all restrict to [lo, TB) (cols < lo contribute 0)
                    js = [2 * g, 2 * g + 1]
                    ds = [j * 128 - q0 for j in js]
                    los = [max(dd, 0) for dd in ds]
                    psQK = psQK_pool.tile([128, 1024], F32, tag="psQK")
                    for h2 in range(2):
                        j, lo = js[h2], los[h2]
                        r = slice(h2 * 64, h2 * 64 + 64)  # alternate row halves
                        nc.tensor.matmul(
                            psQK[:, h2 * 512 + lo : (h2 + 1) * 512],
                            kT2_s[r, j * 128 : (j + 1) * 128],
                            qT2_s[r, q0 + lo : q0 + TB],
                            start=True, stop=True,
                        )
                    # scores are raw q.k here; the 1/sqrt(C) softmax scale is
                    # applied through ACT's free affine pre-scale (keeping it
                    # out of Wq avoids fp16-subnormal weights)
                    pT = pt_pool.tile([128, 1024], F16)
                    if los[0] == 0 and los[1] == 0:
                        nc.scalar.activation(
                            pT[:], psQK[:], mybir.ActivationFunctionType.Exp,
                            scale=float(1.0 / np.sqrt(C)),
                        )
                    else:
                        for h2 in range(2):
                            lo = los[h2]
                            sl = slice(h2 * 512 + lo, (h2 + 1) * 512)
                            nc.scalar.activation(
                                pT[:, sl], psQK[:, sl],
                                mybir.ActivationFunctionType.Exp,
                                scale=float(1.0 / np.sqrt(C)),
                            )
                    for h2 in range(2):
                        j, d, lo = js[h2], ds[h2], los[h2]
                        pj = pT[:, h2 * 512 : (h2 + 1) * 512]
                        if d >= 0:  # diagonal chunk: triangular causal mask
                            nc.vector.tensor_mul(
                                pj[:, d : d + 128], pj[:, d : d + 128], tri_s[:]
                            )
                        nc.tensor.matmul(
                            psO[:, lo:TB],
                            v_s[:, j * HA : (j + 1) * HA],
                            pj[:, lo:TB],
                            start=(j == 0), stop=(j == nsc_i - 1),
                        )
                        if i == NBLK - 1 and j >= nsc_i - 4:
                            # final block: psO cols [128p, 128p+128) take their
                            # last contribution from PV chunk j=12+p — drain
                            # each stripe immediately so the out DMA pipeline
                            # overlaps the remaining PVs instead of the exit
                            # barrier
                            p = j - (nsc_i - 4)
                            sl = slice(p * 128, (p + 1) * 128)
                            nc.vector.tensor_copy(out_sb_last[:, sl], psO[:, sl])
                            nc.sync.dma_start(
                                outT_d[:, q0 + p * 128 : q0 + (p + 1) * 128],
                                out_sb_last[:, sl],
                            )
                if i < NBLK - 1:
                    out_sb = out_pool.tile([HA, TB], F32)
                    # ACT copy (DVE stays free); SWDGE DMA so the sync HWDGE
                    # ring isn't head-of-line-blocked for the x stream
                    nc.scalar.copy(out_sb[:], psO[:])
                    nc.gpsimd.dma_start(outT_d[:, q0 : q0 + TB], out_sb[:])

    nc.compile()
    return nc


def _get_nc():
    if "nc" not in _compiled:
        _compiled["nc"] = build_nc()
    return _compiled["nc"]


def make_in_maps(x, Wk, Wq, Wv):
    x = np.asarray(x, dtype=np.float32)
    Wk = np.asarray(Wk, dtype=np.float32)
    Wq = np.asarray(Wq, dtype=np.float32)
    Wv = np.asarray(Wv, dtype=np.float32)
    # raw Wq (no 1/sqrt(C) here — that scale rides the exp's affine pre-scale)
    wqk = np.concatenate([Wq, Wk], axis=1).astype(np.float16)  # [C, 128]
    wvd = np.concatenate([Wv, Wv], axis=1).astype(np.float16)  # [C, 128] dup
    tri = np.ones((128, 129), dtype=np.float16)
    tri[:, 0:128] = np.triu(np.ones((128, 128), dtype=np.float16))
    in_maps = []
    for b in range(B):
        in_maps.append(
            {
                "xT": np.ascontiguousarray(x[b].T.astype(np.float16)),
                "wqk": wqk,
                "wv": wvd,
                "tri": tri,
            }
        )
    return in_maps


def postprocess(results):
    outs = []
    for b in range(B):
        outT = results[b]["outT"]  # [65, T]
        out = (outT[:H] / outT[H : H + 1]).T  # [T, H]
        outs.append(out)
    return np.stack(outs).astype(np.float32)


def run(x, Wk, Wq, Wv, trace=False, **kw):
    nc = _get_nc()
    in_maps = make_in_maps(x, Wk, Wq, Wv)
    res = run_bass_kernel_spmd(
        nc, in_maps, core_ids=list(range(B)), trace=trace, **kw
    )
    return postprocess(res.results), res


def kernel(x, Wk, Wq, Wv):
    out, _ = run(x, Wk, Wq, Wv, trace=False)
    return out


# revision 33
# speedup vs baseline: 1.1914x; 1.0290x over previous
"""Causal single-head attention (B=8, T=2048, C=1024, H=64) on 8 trn2 NeuronCores.

Strategy (data-parallel over batch, one batch element per core):
  host: feed xT = x[b].T (so C is the partition/contraction dim on chip),
        wqk = [Wq/sqrt(C) | Wk] fused projection weight, wv = [Wv | Wv] (dup for
        column-tiled V projection).
  core, per q-block of 512 tokens:
    passA: one fused matmul chain per C-chunk -> psum rows 0:64 = qT block,
           rows 64:128 = kT block.  Copied (4x) into row-duplicated SBUF tiles
           qT2/kT2 so QK stationaries can alternate PE row-halves.
    passB: column-tiled pair (tile_position (0,0)/(0,64)) -> vT block; PE
           transpose -> v[s, 64]; ones column appended (softmax denominator
           accumulates through the PV matmul's 65th output row).
    QK:   scores^T[s,q] per 128-wide s-chunk; stationary kT chunk alternates
          row-half 0/1 with chunk parity so each LDWEIGHTS overlaps the
          previous chunk's MATMUL (no row-group conflict).
    exp:  ACT reads two adjacent psum banks [128,1024] at once -> pT (fp32r).
          No max subtraction: |scores| < ~1 so exp is exact-safe and equals
          softmax numerator exactly.
    mask: zero cols < d, triangular 128x128 multiply on the diagonal square.
    PV:   out_aug^T[65, q] += v_aug^T-stationary @ pT-moving.
  host: out = (out_aug[:64] / out_aug[64]).T, stack cores.

All matmuls in float32r: full PE rate at moving-N>=256, ~2e-4 rel err.
"""

import numpy as np

import concourse.bass as bass
import concourse.mybir as mybir
import concourse.tile as tile
from concourse import bacc
from concourse.bass_utils import run_bass_kernel_spmd

B, T, C, H = 8, 2048, 1024, 64
TB = 512                 # q-block width
NBLK = T // TB           # 4 q-blocks
NC = C // 128            # 8 contraction chunks
NSC = T // 128           # 16 s-chunks
HA = H + 1               # v augmented with ones column
F32 = mybir.dt.float32
F16 = mybir.dt.float16

_compiled = {}


def build_nc():
    nc = bacc.Bacc("TRN2", target_bir_lowering=False, debug=False, num_devices=8)

    xT_d = nc.dram_tensor("xT", [C, T], F16, kind="ExternalInput").ap()
    wqk_d = nc.dram_tensor("wqk", [C, 128], F16, kind="ExternalInput").ap()
    wv_d = nc.dram_tensor("wv", [C, 128], F16, kind="ExternalInput").ap()
    # col 0:128 = causal upper-triangle mask, col 128 = ones
    tri_d = nc.dram_tensor("tri", [128, 129], F16, kind="ExternalInput").ap()
    outT_d = nc.dram_tensor("outT", [HA, T], F32, kind="ExternalOutput").ap()

    xT_r = xT_d.rearrange("(co ci) t -> ci co t", ci=128)
    wqk_r = wqk_d.rearrange("(co ci) m -> ci co m", ci=128)
    wv_r = wv_d.rearrange("(co ci) m -> ci co m", ci=128)

    with tile.TileContext(nc) as tc:
        with (
            tc.tile_pool(name="const", bufs=1) as cpool,
            tc.tile_pool(name="persist", bufs=1) as ppool,
            tc.tile_pool(name="xin", bufs=24) as xpool,
            tc.tile_pool(name="ptile", bufs=4) as pt_pool,
            tc.tile_pool(name="vtmp", bufs=2) as vtmp_pool,
            tc.tile_pool(name="outsb", bufs=2) as out_pool,
            tc.tile_pool(name="psA", bufs=2, space="PSUM") as psA_pool,
            tc.tile_pool(name="psB", bufs=1, space="PSUM") as psB_pool,
            tc.tile_pool(name="psQK", bufs=2, space="PSUM") as psQK_pool,
            tc.tile_pool(name="psO", bufs=1, space="PSUM") as psO_pool,
        ):
            # constants / weights on the scalar HWDGE ring so they don't queue
            # behind the x stream on the sync ring
            wqk_s = cpool.tile([128, NC, 128], F16)
            wv_s = cpool.tile([128, NC, 128], F16)
            tri_full = cpool.tile([128, 129], F16)
            tri_s = tri_full[:, 0:128]
            ones_s = tri_full[:, 128:129]
            ident = cpool.tile([128, 64], F16)
            # chunk 0 first so the very first matmul's weights land ASAP
            nc.scalar.dma_start(wqk_s[:, 0:1, :], wqk_r[:, 0:1, :])
            nc.scalar.dma_start(wqk_s[:, 1:NC, :], wqk_r[:, 1:NC, :])
            nc.scalar.dma_start(wv_s[:], wv_r[:])
            nc.scalar.dma_start(tri_full[:], tri_d[:])

            # PE warm-up: dummy self-contained matmuls with no DMA deps keep
            # the PE busy through the initial DMA wait so the HAM clock gate
            # reaches K=8/8 before real work arrives.
            warm_w = cpool.tile([128, 128], F32)
            warm_x = cpool.tile([128, 512], F32)
            nc.vector.memset(warm_w[:], 0.0)
            nc.vector.memset(warm_x[:], 0.0)

            def warm_mm(n):
                ps_warm = psQK_pool.tile([128, 512], F32, tag="psQK")
                nc.tensor.matmul(ps_warm[:, 0:n], warm_w[:], warm_x[:, 0:n],
                                 start=True, stop=True)

            # bridge the initial DMA wait (fp32 = 2 passes, long dense ops)
            for w in range(2):
                warm_mm(512)
            for h2 in range(2):
                sl = ident[h2 * 64 : (h2 + 1) * 64, :]
                nc.gpsimd.memset(sl, 0.0)
                nc.gpsimd.affine_select(
                    out=sl,
                    in_=sl,
                    compare_op=mybir.AluOpType.not_equal,
                    fill=1.0,
                    base=0,
                    pattern=[[-1, 64]],
                    channel_multiplier=1,
                )

            # row-duplicated q/k (rows 0:64 == rows 64:128) for row-half
            # alternating QK stationaries/movings
            qT2_s = ppool.tile([128, T], F16)
            kT2_s = ppool.tile([128, T], F16)
            v_s = ppool.tile([128, NSC * HA], F16)

            for i in range(NBLK):
                q0 = i * TB
                # ---- x DMA for this block (sync ring) ----
                x_c = []
                for c in range(NC):
                    xc = xpool.tile([128, TB], F16)
                    if i == 0 and c == 0:
                        # split so the first 128 cols land (and the first
                        # matmul can start) as early as possible
                        nc.sync.dma_start(xc[:, 0:128], xT_r[:, c, q0 : q0 + 128])
                        nc.sync.dma_start(
                            xc[:, 128:TB], xT_r[:, c, q0 + 128 : q0 + TB]
                        )
                    else:
                        nc.sync.dma_start(xc[:], xT_r[:, c, q0 : q0 + TB])
                    x_c.append(xc)

                # ---- passA+passB interleaved per C-chunk (q/k and v
                # projections both consume x_c right as it lands; the pair of
                # matmuls keeps the PE dense inside the DMA-paced window) ----
                psA = psA_pool.tile([128, TB], F32)
                psB = psB_pool.tile([64, TB], F32)
                for c in range(NC):
                    if i == 0 and c == 0:
                        # start=True clears the whole bank; the second piece
                        # (start=False) overwrites its untouched columns
                        nc.tensor.matmul(
                            psA[:, 0:128], wqk_s[:, 0, :], x_c[0][:, 0:128],
                            start=True, stop=False,
                        )
                        nc.tensor.matmul(
                            psA[:, 128:TB], wqk_s[:, 0, :], x_c[0][:, 128:TB],
                            start=False, stop=False,
                        )
                    else:
                        nc.tensor.matmul(
                            psA[:], wqk_s[:, c, :], x_c[c][:],
                            start=(c == 0 and i != 0), stop=(c == NC - 1),
                        )
                    nc.tensor.matmul(
                        psB[:], wv_s[:, c, 0:64], x_c[c][:],
                        start=(c == 0), stop=(c == NC - 1),
                    )
                for h2 in range(2):
                    r = slice(h2 * 64, h2 * 64 + 64)
                    nc.vector.tensor_copy(qT2_s[r, q0 : q0 + TB], psA[0:64, :])
                    nc.vector.tensor_copy(kT2_s[r, q0 : q0 + TB], psA[64:128, :])

                vT_tmp = vtmp_pool.tile([64, TB], F16)
                nc.vector.tensor_copy(vT_tmp[:], psB[:])
                for j4 in range(TB // 128):
                    sj = (TB // 128) * i + j4
                    ps_vt = psQK_pool.tile([128, 64], F16, tag="psQK")
                    nc.tensor.transpose(
                        ps_vt[:],
                        vT_tmp[:, j4 * 128 : (j4 + 1) * 128],
                        ident[0:64, :],
                    )
                    nc.vector.tensor_copy(v_s[:, sj * HA : sj * HA + H], ps_vt[:])
                    nc.vector.tensor_copy(
                        v_s[:, sj * HA + H : sj * HA + HA], ones_s[:]
                    )

                # ---- attention for this q-block ----
                nsc_i = (TB // 128) * (i + 1)  # s-chunks 0..nsc_i-1 (causal)
                psO = psO_pool.tile([HA, TB], F32)
                if i == NBLK - 1:
                    out_sb_last = out_pool.tile([HA, TB], F32)
                for g in range(nsc_i // 2):  # pairs of s-chunks share one exp
                    # lo = first causally-valid q column for each chunk;
                    # QK/exp/PV all restrict to [lo, TB) (cols < lo contribute 0)
                    js = [2 * g, 2 * g + 1]
                    ds = [j * 128 - q0 for j in js]
                    los = [max(dd, 0) for dd in ds]
                    # QK always full-width (cols < lo give garbage-but-finite
                    # scores that PV never reads) so one exp covers the pair
                    psQK = psQK_pool.tile([128, 1024], F32, tag="psQK")
                    for h2 in range(2):
                        j = js[h2]
                        r = slice(h2 * 64, h2 * 64 + 64)  # alternate row halves
                        nc.tensor.matmul(
                            psQK[:, h2 * 512 : (h2 + 1) * 512],
                            kT2_s[r, j * 128 : (j + 1) * 128],
                            qT2_s[r, q0 : q0 + TB],
                            start=True, stop=True,
                        )
                    # scores are raw q.k here; the 1/sqrt(C) softmax scale is
                    # applied through ACT's free affine pre-scale (keeping it
                    # out of Wq avoids fp16-subnormal weights)
                    pT = pt_pool.tile([128, 1024], F16)
                    nc.scalar.activation(
                        pT[:], psQK[:], mybir.ActivationFunctionType.Exp,
                        scale=float(1.0 / np.sqrt(C)),
                    )
                    for h2 in range(2):
                        j, d, lo = js[h2], ds[h2], los[h2]
                        pj = pT[:, h2 * 512 : (h2 + 1) * 512]
                        if d >= 0:  # diagonal chunk: triangular causal mask
                            nc.vector.tensor_mul(
                                pj[:, d : d + 128], pj[:, d : d + 128], tri_s[:]
                            )
                        nc.tensor.matmul(
                            psO[:, lo:TB],
                            v_s[:, j * HA : (j + 1) * HA],
                            pj[:, lo:TB],
                            start=(j == 0), stop=(j == nsc_i - 1),
                        )
                        if i == NBLK - 1 and j >= nsc_i - 4:
                            # final block: psO cols [128p, 128p+128) take their
                            # last contribution from PV chunk j=12+p — drain
                            # each stripe immediately so the out DMA pipeline
                            # overlaps the remaining PVs instead of the exit
                            # barrier
                            p = j - (nsc_i - 4)
                            sl = slice(p * 128, (p + 1) * 128)
                            nc.vector.tensor_copy(out_sb_last[:, sl], psO[:, sl])
                            nc.sync.dma_start(
                                outT_d[:, q0 + p * 128 : q0 + (p + 1) * 128],
                                out_sb_last[:, sl],
                            )
                if i < NBLK - 1:
                    out_sb = out_pool.tile([HA, TB], F32)
                    # ACT copy (DVE stays free); SWDGE DMA so the sync HWDGE
                    # ring isn't head-of-line-blocked for the x stream
                    nc.scalar.copy(out_sb[:], psO[:])
                    nc.gpsimd.dma_start(outT_d[:, q0 : q0 + TB], out_sb[:])

    nc.compile()
    return nc


def _get_nc():
    if "nc" not in _compiled:
        _compiled["nc"] = build_nc()
    return _compiled["nc"]


def make_in_maps(x, Wk, Wq, Wv):
    x = np.asarray(x, dtype=np.float32)
    Wk = np.asarray(Wk, dtype=np.float32)
    Wq = np.asarray(Wq, dtype=np.float32)
    Wv = np.asarray(Wv, dtype=np.float32)
    # raw Wq (no 1/sqrt(C) here — that scale rides the exp's affine pre-scale)
    wqk = np.concatenate([Wq, Wk], axis=1).astype(np.float16)  # [C, 128]
    wvd = np.concatenate([Wv, Wv], axis=1).astype(np.float16)  # [C, 128] dup
    tri = np.ones((128, 129), dtype=np.float16)
    tri[:, 0:128] = np.triu(np.ones((128, 128), dtype=np.float16))
    in_maps = []
    for b in range(B):
        in_maps.append(
            {
                "xT": np.ascontiguousarray(x[b].T.astype(np.float16)),
                "wqk": wqk,
                "wv": wvd,
                "tri": tri,
            }
        )
    return in_maps


def postprocess(results):
    outs = []
    for b in range(B):
        outT = results[b]["outT"]  # [65, T]
        out = (outT[:H] / outT[H : H + 1]).T  # [T, H]
        outs.append(out)
    return np.stack(outs).astype(np.float32)


def run(x, Wk, Wq, Wv, trace=False, **kw):
    nc = _get_nc()
    in_maps = make_in_maps(x, Wk, Wq, Wv)
    res = run_bass_kernel_spmd(
        nc, in_maps, core_ids=list(range(B)), trace=trace, **kw
    )
    return postprocess(res.results), res


def kernel(x, Wk, Wq, Wv):
    out, _ = run(x, Wk, Wq, Wv, trace=False)
    return out


# revision 35
# speedup vs baseline: 1.1929x; 1.0013x over previous
"""Causal single-head attention (B=8, T=2048, C=1024, H=64) on 8 trn2 NeuronCores.

Strategy (data-parallel over batch, one batch element per core):
  host: feed xT = x[b].T in fp16 (C becomes the on-chip contraction/partition
        dim and the dominant DMA halves), wqk = [Wq | Wk] fused projection
        weight, wv = [Wv | Wv].
  core, per q-block of 512 tokens:
    proj: per C-chunk, a fused [Wq|Wk] matmul (psum rows 0:64 = qT block,
          rows 64:128 = kT block) interleaved with the Wv matmul so both ride
          the DMA-paced window.  qT/kT copied into row-duplicated SBUF tiles
          so QK stationaries/movings can alternate PE row-halves.
    vT -> v: PE transposes + ones column appended (the softmax denominator
          accumulates through the PV matmul's 65th output row).
    QK:   scores^T[s,q] per 128-wide s-chunk; chunk parity alternates PE
          row-halves so each self-loading matmul's weight load overlaps the
          previous chunk's matmul (no row-group conflict) and pairs issue
          concurrently.
    exp:  one ACT op per chunk-pair reads two adjacent psum banks [128,1024]
          -> pT (fp16), with the 1/sqrt(C) softmax scale applied via ACT's
          free affine pre-scale (keeping it out of Wq avoids fp16-subnormal
          weights).  No max subtraction: |scores/sqrt(C)| < ~1 so exp is
          overflow-safe and equals the softmax numerator exactly.
    mask: triangular 128x128 multiply on the diagonal square; columns left of
          the causal frontier are simply never read by PV.
    PV:   out_aug^T[65, q] += v_aug-stationary @ pT-moving (causally-partial
          widths).  Final block drains psO stripe-by-stripe as each stripe's
          last PV lands, overlapping the out DMA with the remaining work.
  host: out = (out_aug[:64] / out_aug[64]).T, stack cores.

fp16 everywhere on the PE (full rate, FWL weight loads, half DMA); all
accumulation in fp32 PSUM.  fp32 warm-up matmuls bridge the initial DMA wait
so the HAM clock gate reaches K=8/8 before real work arrives.  End-to-end
absmax/scale error vs the fp32 reference: ~4.7e-4.
"""

import numpy as np

import concourse.bass as bass
import concourse.mybir as mybir
import concourse.tile as tile
from concourse import bacc
from concourse.bass_utils import run_bass_kernel_spmd

B, T, C, H = 8, 2048, 1024, 64
TB = 512                 # q-block width
NBLK = T // TB           # 4 q-blocks
NC = C // 128            # 8 contraction chunks
NSC = T // 128           # 16 s-chunks
HA = H + 1               # v augmented with ones column
F32 = mybir.dt.float32
F16 = mybir.dt.float16

_compiled = {}


def build_nc():
    nc = bacc.Bacc("TRN2", target_bir_lowering=False, debug=False, num_devices=8)

    xT_d = nc.dram_tensor("xT", [C, T], F16, kind="ExternalInput").ap()
    wqk_d = nc.dram_tensor("wqk", [C, 128], F16, kind="ExternalInput").ap()
    wv_d = nc.dram_tensor("wv", [C, 128], F16, kind="ExternalInput").ap()
    # col 0:128 = causal upper-triangle mask, col 128 = ones
    tri_d = nc.dram_tensor("tri", [128, 129], F16, kind="ExternalInput").ap()
    outT_d = nc.dram_tensor("outT", [HA, T], F32, kind="ExternalOutput").ap()

    xT_r = xT_d.rearrange("(co ci) t -> ci co t", ci=128)
    wqk_r = wqk_d.rearrange("(co ci) m -> ci co m", ci=128)
    wv_r = wv_d.rearrange("(co ci) m -> ci co m", ci=128)

    with tile.TileContext(nc) as tc:
        with (
            tc.tile_pool(name="const", bufs=1) as cpool,
            tc.tile_pool(name="persist", bufs=1) as ppool,
            tc.tile_pool(name="xin", bufs=32) as xpool,
            tc.tile_pool(name="ptile", bufs=4) as pt_pool,
            tc.tile_pool(name="vtmp", bufs=2) as vtmp_pool,
            tc.tile_pool(name="outsb", bufs=2) as out_pool,
            tc.tile_pool(name="psA", bufs=2, space="PSUM") as psA_pool,
            tc.tile_pool(name="psB", bufs=1, space="PSUM") as psB_pool,
            tc.tile_pool(name="psQK", bufs=2, space="PSUM") as psQK_pool,
            tc.tile_pool(name="psO", bufs=1, space="PSUM") as psO_pool,
        ):
            # constants / weights on the scalar HWDGE ring so they don't queue
            # behind the x stream on the sync ring
            wqk_s = cpool.tile([128, NC, 128], F16)
            wv_s = cpool.tile([128, NC, 128], F16)
            tri_full = cpool.tile([128, 129], F16)
            tri_s = tri_full[:, 0:128]
            ones_s = tri_full[:, 128:129]
            ident = cpool.tile([128, 64], F16)
            # chunk 0 first so the very first matmul's weights land ASAP
            nc.scalar.dma_start(wqk_s[:, 0:1, :], wqk_r[:, 0:1, :])
            nc.scalar.dma_start(wqk_s[:, 1:NC, :], wqk_r[:, 1:NC, :])
            nc.scalar.dma_start(wv_s[:], wv_r[:])
            nc.scalar.dma_start(tri_full[:], tri_d[:])

            # PE warm-up: dummy self-contained matmuls with no DMA deps keep
            # the PE busy through the initial DMA wait so the HAM clock gate
            # reaches K=8/8 before real work arrives.
            warm_w = cpool.tile([128, 128], F32)
            warm_x = cpool.tile([128, 512], F32)
            nc.vector.memset(warm_w[:], 0.0)
            nc.vector.memset(warm_x[:], 0.0)

            def warm_mm(n):
                ps_warm = psQK_pool.tile([128, 512], F32, tag="psQK")
                nc.tensor.matmul(ps_warm[:, 0:n], warm_w[:], warm_x[:, 0:n],
                                 start=True, stop=True)

            # bridge the initial DMA wait (fp32 = 2 passes, long dense ops)
            for w in range(2):
                warm_mm(512)
            for h2 in range(2):
                sl = ident[h2 * 64 : (h2 + 1) * 64, :]
                nc.gpsimd.memset(sl, 0.0)
                nc.gpsimd.affine_select(
                    out=sl,
                    in_=sl,
                    compare_op=mybir.AluOpType.not_equal,
                    fill=1.0,
                    base=0,
                    pattern=[[-1, 64]],
                    channel_multiplier=1,
                )

            # row-duplicated q/k (rows 0:64 == rows 64:128) for row-half
            # alternating QK stationaries/movings
            qT2_s = ppool.tile([128, T], F16)
            kT2_s = ppool.tile([128, T], F16)
            v_s = ppool.tile([128, NSC * HA], F16)

            for i in range(NBLK):
                q0 = i * TB
                # ---- x DMA for this block (sync ring) ----
                x_c = []
                for c in range(NC):
                    xc = xpool.tile([128, TB], F16)
                    if i == 0 and c == 0:
                        # split so the first 128 cols land (and the first
                        # matmul can start) as early as possible
                        nc.sync.dma_start(xc[:, 0:128], xT_r[:, c, q0 : q0 + 128])
                        nc.sync.dma_start(
                            xc[:, 128:TB], xT_r[:, c, q0 + 128 : q0 + TB]
                        )
                    else:
                        nc.sync.dma_start(xc[:], xT_r[:, c, q0 : q0 + TB])
                    x_c.append(xc)

                # ---- passA+passB interleaved per C-chunk (q/k and v
                # projections both consume x_c right as it lands; the pair of
                # matmuls keeps the PE dense inside the DMA-paced window) ----
                psA = psA_pool.tile([128, TB], F32)
                psB = psB_pool.tile([64, TB], F32)
                for c in range(NC):
                    if i == 0 and c == 0:
                        # start=True clears the whole bank; the second piece
                        # (start=False) overwrites its untouched columns
                        nc.tensor.matmul(
                            psA[:, 0:128], wqk_s[:, 0, :], x_c[0][:, 0:128],
                            start=True, stop=False,
                        )
                        nc.tensor.matmul(
                            psA[:, 128:TB], wqk_s[:, 0, :], x_c[0][:, 128:TB],
                            start=False, stop=False,
                        )
                    else:
                        nc.tensor.matmul(
                            psA[:], wqk_s[:, c, :], x_c[c][:],
                            start=(c == 0 and i != 0), stop=(c == NC - 1),
                        )
                    nc.tensor.matmul(
                        psB[:], wv_s[:, c, 0:64], x_c[c][:],
                        start=(c == 0), stop=(c == NC - 1),
                    )
                for h2 in range(2):
                    r = slice(h2 * 64, h2 * 64 + 64)
                    nc.vector.tensor_copy(qT2_s[r, q0 : q0 + TB], psA[0:64, :])
                    nc.vector.tensor_copy(kT2_s[r, q0 : q0 + TB], psA[64:128, :])

                vT_tmp = vtmp_pool.tile([64, TB], F16)
                nc.vector.tensor_copy(vT_tmp[:], psB[:])
                for j4 in range(TB // 128):
                    sj = (TB // 128) * i + j4
                    ps_vt = psQK_pool.tile([128, 64], F16, tag="psQK")
                    nc.tensor.transpose(
                        ps_vt[:],
                        vT_tmp[:, j4 * 128 : (j4 + 1) * 128],
                        ident[0:64, :],
                    )
                    nc.vector.tensor_copy(v_s[:, sj * HA : sj * HA + H], ps_vt[:])
                    nc.vector.tensor_copy(
                        v_s[:, sj * HA + H : sj * HA + HA], ones_s[:]
                    )

                # ---- attention for this q-block ----
                nsc_i = (TB // 128) * (i + 1)  # s-chunks 0..nsc_i-1 (causal)
                psO = psO_pool.tile([HA, TB], F32)
                if i == NBLK - 1:
                    out_sb_last = out_pool.tile([HA, TB], F32)
                for g in range(nsc_i // 2):  # pairs of s-chunks share one exp
                    # lo = first causally-valid q column for each chunk;
                    # QK/exp/PV all restrict to [lo, TB) (cols < lo contribute 0)
                    js = [2 * g, 2 * g + 1]
                    ds = [j * 128 - q0 for j in js]
                    los = [max(dd, 0) for dd in ds]
                    # QK always full-width (cols < lo give garbage-but-finite
                    # scores that PV never reads) so one exp covers the pair
                    psQK = psQK_pool.tile([128, 1024], F32, tag="psQK")
                    for h2 in range(2):
                        j = js[h2]
                        r = slice(h2 * 64, h2 * 64 + 64)  # alternate row halves
                        nc.tensor.matmul(
                            psQK[:, h2 * 512 : (h2 + 1) * 512],
                            kT2_s[r, j * 128 : (j + 1) * 128],
                            qT2_s[r, q0 : q0 + TB],
                            start=True, stop=True,
                        )
                    # scores are raw q.k here; the 1/sqrt(C) softmax scale is
                    # applied through ACT's free affine pre-scale (keeping it
                    # out of Wq avoids fp16-subnormal weights)
                    pT = pt_pool.tile([128, 1024], F16)
                    nc.scalar.activation(
                        pT[:], psQK[:], mybir.ActivationFunctionType.Exp,
                        scale=float(1.0 / np.sqrt(C)),
                    )
                    for h2 in range(2):
                        j, d, lo = js[h2], ds[h2], los[h2]
                        pj = pT[:, h2 * 512 : (h2 + 1) * 512]
                        if d >= 0:  # diagonal chunk: triangular causal mask
                            nc.vector.tensor_mul(
                                pj[:, d : d + 128], pj[:, d : d + 128], tri_s[:]
                            )
                        nc.tensor.matmul(
                            psO[:, lo:TB],
                            v_s[:, j * HA : (j + 1) * HA],
                            pj[:, lo:TB],
                            start=(j == 0), stop=(j == nsc_i - 1),
                        )
                        if i == NBLK - 1 and j >= nsc_i - 4:
                            # final block: psO cols [128p, 128p+128) take their
                            # last contribution from PV chunk j=12+p — drain
                            # each stripe immediately so the out DMA pipeline
                            # overlaps the remaining PVs instead of the exit
                            # barrier
                            p = j - (nsc_i - 4)
                            sl = slice(p * 128, (p + 1) * 128)
                            nc.vector.tensor_copy(out_sb_last[:, sl], psO[:, sl])
                            nc.sync.dma_start(
                                outT_d[:, q0 + p * 128 : q0 + (p + 1) * 128],
                                out_sb_last[:, sl],
                            )
                if i < NBLK - 1:
                    out_sb = out_pool.tile([HA, TB], F32)
                    # ACT copy (DVE stays free); SWDGE DMA so the sync HWDGE
                    # ring isn't head-of-line-blocked for the x stream
                    nc.scalar.copy(out_sb[:], psO[:])
                    nc.gpsimd.dma_start(outT_d[:, q0 : q0 + TB], out_sb[:])

    nc.compile()
    return nc


def _get_nc():
    if "nc" not in _compiled:
        _compiled["nc"] = build_nc()
    return _compiled["nc"]


def make_in_maps(x, Wk, Wq, Wv):
    x = np.asarray(x, dtype=np.float32)
    Wk = np.asarray(Wk, dtype=np.float32)
    Wq = np.asarray(Wq, dtype=np.float32)
    Wv = np.asarray(Wv, dtype=np.float32)
    # raw Wq (no 1/sqrt(C) here — that scale rides the exp's affine pre-scale)
    wqk = np.concatenate([Wq, Wk], axis=1).astype(np.float16)  # [C, 128]
    wvd = np.concatenate([Wv, Wv], axis=1).astype(np.float16)  # [C, 128] dup
    tri = np.ones((128, 129), dtype=np.float16)
    tri[:, 0:128] = np.triu(np.ones((128, 128), dtype=np.float16))
    in_maps = []
    for b in range(B):
        in_maps.append(
            {
                "xT": np.ascontiguousarray(x[b].T.astype(np.float16)),
                "wqk": wqk,
                "wv": wvd,
                "tri": tri,
            }
        )
    return in_maps


def postprocess(results):
    outs = []
    for b in range(B):
        outT = results[b]["outT"]  # [65, T]
        out = (outT[:H] / outT[H : H + 1]).T  # [T, H]
        outs.append(out)
    return np.stack(outs).astype(np.float32)


def run(x, Wk, Wq, Wv, trace=False, **kw):
    nc = _get_nc()
    in_maps = make_in_maps(x, Wk, Wq, Wv)
    res = run_bass_kernel_spmd(
        nc, in_maps, core_ids=list(range(B)), trace=trace, **kw
    )
    return postprocess(res.results), res


def kernel(x, Wk, Wq, Wv):
    out, _ = run(x, Wk, Wq, Wv, trace=False)
    return out
